# revision 2
# baseline (speedup 1.0000x reference)
"""Circular shift kernel for Trainium2 (Bass), SPMD over 8 NeuronCores.

Reference semantics: out = vec @ roll(eye(d), -1, axis=0), i.e.
out[b, j] = vec[b, (j-1) mod d] -- a roll by +1 along the last axis.

Sharding: data-parallel along the batch axis; each core handles a
contiguous [1024, 4096] row block and rolls locally with direct
DRAM->DRAM DMA copies (each byte passes an SDMA engine once; the
per-core moved-bytes cap is ~333 GB/s across the 16 engines).

Structure (per core), all on the SP HWDGE queue except the wraps:

  bulk: the flat shifted copy out_flat[4096:] = in_flat[4095:-1],
    issued as 16 instructions grouped into 4 contiguous quarters.
    Descriptor spray is round-robin per instruction starting at engine
    64, so instruction sizes control per-engine load:
      - 13 x A (16*16368 elems): 16 descs of 65472 B, one per engine
      - 3 x B (15*16369 elems, odd so the balancer's factor-16 split
        fails): 15 descs of 65476 B on engines 64-78 ONLY
      - 1 x C (49059 elems): 9 descs of 21804 B on engines 64-72
    => engine 79 gets 13 descriptors instead of 16. Engine 79 is
    intermittently ~16-20% slower (periodic slow packets, ~half of
    runs); underloading it removes an ~8 us straggler tail at no cost
    to the other engines.

  head: out[0, 1:4096] = in[0, 0:4095] (15 small descs).

  wraps (scalar/Activation HWDGE queue): out[r, 0] = in[r, 4095] as
    4-B descriptors. A wrap for rows [r0, r1) launches as soon as the
    bulk quarters covering those rows complete (semaphore-gated), so
    the first three wraps hide under the remaining bulk and only the
    last (~1 us) is exposed. The 4-B writes share 64-B beats with bulk
    writes of the SAME rows, which race-corrupts if concurrent
    (measured: ~76 corrupted elems) -- the row-disjoint pipelining
    avoids that while still overlapping.
"""

import numpy as np

N_CORES = 8
ROWS = 8192
COLS = 4096
SHARD_ROWS = ROWS // N_CORES  # 1024
N = SHARD_ROWS * COLS

A_SZ = 16 * 16368  # -> 16 descs of 65472 B (all engines)
B_SZ = 15 * 16369  # -> 15 descs of 65476 B (engines 64-78)
C_SZ = 49059       # -> 9 descs of 21804 B (engines 64-72)

GROUPS = [
    [A_SZ] * 4,
    [A_SZ] * 4,
    [A_SZ] * 4,
    [A_SZ, B_SZ, B_SZ, B_SZ, C_SZ],
]
assert sum(sum(g) for g in GROUPS) == N - COLS


def _build_nc():
    import concourse.bass as bass
    import concourse.mybir as mybir

    nc = bass.Bass("TRN2", monotonic_sem_count=0, enable_partition_id=False)
    x = nc.dram_tensor(
        "vec", [SHARD_ROWS, COLS], mybir.dt.float32, kind="ExternalInput"
    )
    y = nc.dram_tensor(
        "out", [SHARD_ROWS, COLS], mybir.dt.float32, kind="ExternalOutput"
    )
    xf = x[:, :].flatten()
    yf = y[:, :].flatten()

    S = [nc.alloc_semaphore(f"S{k}") for k in range(len(GROUPS))]
    W = nc.alloc_semaphore("W")

    pos = COLS
    bounds = []
    first = True
    for k, g in enumerate(GROUPS):
        for sz in g:
            nc.sync.dma_start(
                out=yf[pos : pos + sz], in_=xf[pos - 1 : pos + sz - 1]
            ).then_inc(S[k], 16)
            pos += sz
            if first:
                nc.sync.dma_start(out=yf[1:COLS], in_=xf[0 : COLS - 1]).then_inc(
                    S[0], 16
                )
                first = False
        bounds.append(pos // COLS)
    assert pos == N and bounds[-1] == SHARD_ROWS
    thresholds = [
        16 * (len(g) + (1 if k == 0 else 0)) for k, g in enumerate(GROUPS)
    ]

    r0 = 0
    need_w = 0
    with nc.allow_non_contiguous_dma(reason="wrap column: 1 elem per row"):
        for k, r1 in enumerate(bounds):
            nc.scalar.wait_ge(S[k], thresholds[k])
            nc.scalar.dma_start(
                out=y[r0:r1, 0:1], in_=x[r0:r1, COLS - 1 : COLS]
            ).then_inc(W, 16)
            need_w += 16
            r0 = r1

    nc.sync.wait_ge(W, need_w)
    for k, s in enumerate(S):
        nc.sync.wait_ge(s, thresholds[k])
    return nc


def run(vec: np.ndarray, **spmd_kwargs):
    """Build + run the SPMD kernel; returns (full_output, BassKernelResults)."""
    from concourse import bass_utils

    vec = np.ascontiguousarray(vec, dtype=np.float32)
    assert vec.shape == (ROWS, COLS), vec.shape
    nc = _build_nc()
    in_maps = [
        {"vec": vec[i * SHARD_ROWS : (i + 1) * SHARD_ROWS]} for i in range(N_CORES)
    ]
    res = bass_utils.run_bass_kernel_spmd(
        nc, in_maps, core_ids=list(range(N_CORES)), **spmd_kwargs
    )
    out = np.concatenate([r["out"] for r in res.results], axis=0)
    return out, res


def kernel(vec: np.ndarray) -> np.ndarray:
    out, _ = run(vec)
    return out


# revision 3
# speedup vs baseline: 1.0170x; 1.0170x over previous
"""Circular shift kernel for Trainium2 (Bass), SPMD over 8 NeuronCores.

Reference semantics: out = vec @ roll(eye(d), -1, axis=0), i.e.
out[b, j] = vec[b, (j-1) mod d] -- a roll by +1 along the last axis.

Sharding: data-parallel along the batch axis; each core handles a
contiguous [1024, 4096] row block and rolls locally with direct
DRAM->DRAM DMA copies (each byte passes an SDMA engine once; the
per-core moved-bytes cap is ~333 GB/s across the 16 engines).

Structure (per core):

  bulk (SP HWDGE queue): the flat shifted copy
    out_flat[4096:] = in_flat[4095:-1], issued as 17 instructions in 5
    region-ordered groups. Descriptor spray is round-robin per
    instruction starting at engine 64, and the AP balancer splits flat
    regions into <=65535-B descriptors preferring a multiple-of-16
    count, so instruction SIZES control per-engine load:
      - 13 x A (16*16368 elems): 16 descs of 65472 B, one per engine
      - 3 x B (15*16369 elems; odd total so the factor-16 split fails):
        15 descs of 65476 B on engines 64-78 ONLY
      - 1 x C (49059 elems): 9 descs of 21804 B on engines 64-72
    => engine 79 gets 13 descriptors instead of 16. Engine 79 is
    intermittently ~16-20% slower (periodic slow packets, roughly half
    of runs); underloading it removes an ~8 us straggler tail at no
    cost to the other engines.

  head: out[0, 1:4096] = in[0, 0:4095] (15 small descs, engines 64-78).

  wraps (scalar/Activation HWDGE queue): out[r, 0] = in[r, 4095] as 4-B
    descriptors, pipelined: the wrap for each group's rows launches as
    soon as that group's bulk completes (semaphore-gated), so all but
    the last wrap hide under the remaining bulk. The final group is the
    small C region, so the exposed last wrap covers only rows
    1011-1023 (13 descs). The 4-B writes share 64-B beats with bulk
    writes of the SAME rows and race-corrupt if concurrent (measured);
    row-disjoint pipelining avoids that while still overlapping.

  Kernel end: a single wait on the wrap semaphore -- wrap completion
  transitively implies all bulk/head DMAs completed (each wrap only
  dispatches after its group's semaphore).
"""

import numpy as np

N_CORES = 8
ROWS = 8192
COLS = 4096
SHARD_ROWS = ROWS // N_CORES  # 1024
N = SHARD_ROWS * COLS

A_SZ = 16 * 16368  # -> 16 descs of 65472 B (all engines)
B_SZ = 15 * 16369  # -> 15 descs of 65476 B (engines 64-78)
C_SZ = 49059       # -> 9 descs of 21804 B (engines 64-72)

GROUPS = [
    [A_SZ] * 4,                # rows ..256
    [A_SZ] * 4,                # rows ..512
    [A_SZ] * 4,                # rows ..768
    [A_SZ, B_SZ, B_SZ, B_SZ],  # rows ..1011
    [C_SZ],                    # rows ..1024 (tiny last group)
]
assert sum(sum(g) for g in GROUPS) == N - COLS


def _build_nc():
    import concourse.bass as bass
    import concourse.mybir as mybir

    nc = bass.Bass("TRN2", monotonic_sem_count=0, enable_partition_id=False)
    x = nc.dram_tensor(
        "vec", [SHARD_ROWS, COLS], mybir.dt.float32, kind="ExternalInput"
    )
    y = nc.dram_tensor(
        "out", [SHARD_ROWS, COLS], mybir.dt.float32, kind="ExternalOutput"
    )
    xf = x[:, :].flatten()
    yf = y[:, :].flatten()

    S = [nc.alloc_semaphore(f"S{k}") for k in range(len(GROUPS))]
    W = nc.alloc_semaphore("W")

    pos = COLS
    bounds = []
    first = True
    for k, g in enumerate(GROUPS):
        for sz in g:
            nc.sync.dma_start(
                out=yf[pos : pos + sz], in_=xf[pos - 1 : pos + sz - 1]
            ).then_inc(S[k], 16)
            pos += sz
            if first:
                nc.sync.dma_start(out=yf[1:COLS], in_=xf[0 : COLS - 1]).then_inc(
                    S[0], 16
                )
                first = False
        bounds.append(pos // COLS)
    assert pos == N and bounds[-1] == SHARD_ROWS
    thresholds = [
        16 * (len(g) + (1 if k == 0 else 0)) for k, g in enumerate(GROUPS)
    ]

    r0 = 0
    need_w = 0
    with nc.allow_non_contiguous_dma(reason="wrap column: 1 elem per row"):
        for k, r1 in enumerate(bounds):
            nc.scalar.wait_ge(S[k], thresholds[k])
            if r1 > r0:
                nc.scalar.dma_start(
                    out=y[r0:r1, 0:1], in_=x[r0:r1, COLS - 1 : COLS]
                ).then_inc(W, 16)
                need_w += 16
                r0 = r1

    nc.sync.wait_ge(W, need_w)
    return nc


def run(vec: np.ndarray, **spmd_kwargs):
    """Build + run the SPMD kernel; returns (full_output, BassKernelResults)."""
    from concourse import bass_utils

    vec = np.ascontiguousarray(vec, dtype=np.float32)
    assert vec.shape == (ROWS, COLS), vec.shape
    nc = _build_nc()
    in_maps = [
        {"vec": vec[i * SHARD_ROWS : (i + 1) * SHARD_ROWS]} for i in range(N_CORES)
    ]
    res = bass_utils.run_bass_kernel_spmd(
        nc, in_maps, core_ids=list(range(N_CORES)), **spmd_kwargs
    )
    out = np.concatenate([r["out"] for r in res.results], axis=0)
    return out, res


def kernel(vec: np.ndarray) -> np.ndarray:
    out, _ = run(vec)
    return out
